# revision 1
# baseline (speedup 1.0000x reference)
"""Causal GQA self-attention (B=2, S=2048, D=2048, 32 Q heads / 8 KV heads,
hd=64, RoPE) on 8 TRN2 NeuronCores — pipelined rewrite.

Sharding: 2-way data parallel over batch x 4-way tensor parallel over heads
(core c: batch c//4, head group c%4 -> 8 Q heads, 2 KV heads).

Key structure (vs the phase-sequential baseline):
 - seq-block-outer pipeline: for each 512-token block sb, project K/V/Q for
   that block, then immediately run attention for q-block j=sb (causal: only
   needs K/V blocks 0..sb). Projection of block sb+1 overlaps attention j=sb.
 - AllGather of the attention output is chunked by seq block and issued right
   after each attention block, overlapping later compute; out-projection
   consumes chunks as they arrive.
 - RoPE rotate-half is a partition permutation; Q/K head dims are permuted
   host-side so the rotation is a within-32-partition stream_shuffle (DVE),
   with the sign folded into the sin table. No PE rotation matmul.
 - V is projected directly in [seq, feat] layout (x-chunk as lhsT), removing
   the transpose pass.
 - elementwise work is spread across ACT (psum->sbuf copies, exp),
   DVE (rope muls/adds, masks, normalization) and Pool (sin-mul, denominator
   partition_broadcast).

Matmuls bf16 x bf16 -> fp32 PSUM; softmax in fp32/bf16 mix.
"""
import sys
sys.path.insert(0, "/opt/trn_rl_repo")
import numpy as np
import ml_dtypes
import concourse.bass as bass
import concourse.mybir as mybir
import concourse.tile as tile
from concourse import bacc
from concourse.bass_utils import run_bass_kernel_spmd

MODEL_DIM = 2048
SEQ = 2048
HEAD_DIM = 64
ROPE_BASE = 10000.0
BATCH = 2
NCORES = 8
GROUPS = [[0, 1, 2, 3], [4, 5, 6, 7]]
QF = 512   # q features per core (8 heads * 64)
KF = 128   # kv features per core (2 kv heads * 64)
NSB = 4    # seq blocks of 512

f32 = mybir.dt.float32
bf16 = mybir.dt.bfloat16
ACTF = mybir.ActivationFunctionType
BF = ml_dtypes.bfloat16

# head-dim permutation that makes rotate-half a within-32 partition shuffle
PERM = np.concatenate([np.arange(0, 16), np.arange(32, 48),
                       np.arange(16, 32), np.arange(48, 64)])
SHUF = list(range(16, 32)) + list(range(16))  # swap 16-halves per quadrant

_cache = {}


def _build_kernel():
    nc = bacc.Bacc(None, target_bir_lowering=False, debug=False,
                   num_devices=NCORES)
    xT = nc.dram_tensor("xT", [MODEL_DIM, SEQ], bf16, kind="ExternalInput").ap()
    wq = nc.dram_tensor("wq", [MODEL_DIM, QF], bf16, kind="ExternalInput").ap()
    wk = nc.dram_tensor("wk", [MODEL_DIM, KF], bf16, kind="ExternalInput").ap()
    wv = nc.dram_tensor("wv", [MODEL_DIM, KF], bf16, kind="ExternalInput").ap()
    wo = nc.dram_tensor("wo", [MODEL_DIM, QF], bf16, kind="ExternalInput").ap()
    cosr = nc.dram_tensor("cosr", [128, SEQ], bf16, kind="ExternalInput").ap()
    sinr = nc.dram_tensor("sinr", [128, SEQ], bf16, kind="ExternalInput").ap()
    masks = nc.dram_tensor("masks", [128, 4 * 512], bf16, kind="ExternalInput").ap()
    out = nc.dram_tensor("out", [SEQ, QF], f32, kind="ExternalOutput").ap()

    with tile.TileContext(nc) as tc:
        from contextlib import ExitStack
        with ExitStack() as ctx:
            consts = ctx.enter_context(tc.tile_pool(name="consts", bufs=1))
            persist = ctx.enter_context(tc.tile_pool(name="persist", bufs=1))
            dram = ctx.enter_context(tc.tile_pool(name="dram", bufs=1, space="DRAM"))

            wq_sb = consts.tile([128, 16, QF], bf16, tag="wq")
            wk_sb = consts.tile([128, 16, KF], bf16, tag="wk")
            wv_sb = consts.tile([128, 16, KF], bf16, tag="wv")
            wo_sb = consts.tile([128, 16, QF], bf16, tag="wo")
            cos_sb = consts.tile([128, SEQ], bf16, tag="cos")
            sin_sb = consts.tile([128, SEQ], bf16, tag="sin")
            masks_sb = consts.tile([128, 4 * 512], bf16, tag="masks")
            nc.sync.dma_start(wq_sb[:], wq.rearrange("(c p) m -> p c m", p=128))
            nc.sync.dma_start(wk_sb[:], wk.rearrange("(c p) m -> p c m", p=128))
            nc.sync.dma_start(wv_sb[:], wv.rearrange("(c p) m -> p c m", p=128))
            nc.sync.dma_start(wo_sb[:], wo.rearrange("(c p) n -> p c n", p=128))
            nc.sync.dma_start(cos_sb[:], cosr[:])
            nc.sync.dma_start(sin_sb[:], sinr[:])
            nc.sync.dma_start(masks_sb[:], masks[:])

            qT = [persist.tile([128, SEQ], bf16, tag=f"qT{i}", name=f"qT{i}")
                  for i in range(4)]
            kT = persist.tile([128, SEQ], bf16, tag="kT")
            kT2 = persist.tile([128, SEQ], bf16, tag="kT2")
            V1 = persist.tile([128, 16, 132], bf16, tag="V1")
            nc.vector.memset(V1[:, :, 64:65], 1.0)    # fused-denominator col kv0
            nc.vector.memset(V1[:, :, 130:131], 1.0)  # fused-denominator col kv1

            y_loc = [dram.tile([QF, 512], bf16, tag=f"yl{j}", name=f"yl{j}")
                     for j in range(NSB)]
            y_ful = [dram.tile([4 * QF, 512], bf16, tag=f"yf{j}", name=f"yf{j}")
                     for j in range(NSB)]

            with tc.tile_pool(name="xts", bufs=2) as xtp, \
                 tc.tile_pool(name="pp", bufs=1, space="PSUM") as pp, \
                 tc.tile_pool(name="psc", bufs=2, space="PSUM") as psc, \
                 tc.tile_pool(name="pavp", bufs=1, space="PSUM") as pavp, \
                 tc.tile_pool(name="rope", bufs=2) as rp, \
                 tc.tile_pool(name="expp", bufs=4) as ep, \
                 tc.tile_pool(name="norm", bufs=2) as nm:

                def rope(src_ps, dst, ssl, k2=False):
                    # rope(z) = z*cos + shuffle(z)*sin' ; sign folded into sin'
                    zb = rp.tile([128, 512], bf16, tag="zb")
                    nc.scalar.activation(zb[:], src_ps[:], ACTF.Copy)
                    zr = rp.tile([128, 512], bf16, tag="zr")
                    nc.vector.stream_shuffle(zr[:], zb[:], SHUF)
                    t1 = rp.tile([128, 512], bf16, tag="t1")
                    nc.vector.tensor_mul(t1[:], zb[:], cos_sb[:, ssl])
                    t2 = rp.tile([128, 512], bf16, tag="t2")
                    nc.gpsimd.tensor_mul(t2[:], zr[:], sin_sb[:, ssl])
                    nc.vector.tensor_add(dst[:, ssl], t1[:], t2[:])
                    if k2:
                        nc.vector.tensor_copy(kT2[0:64, ssl], dst[64:128, ssl])
                        nc.vector.tensor_copy(kT2[64:128, ssl], dst[0:64, ssl])

                for sb in range(NSB):
                    ssl = slice(sb * 512, (sb + 1) * 512)
                    # ---- projection of seq block sb ----
                    xts = xtp.tile([128, 16, 512], bf16, tag="xts")
                    for dc in range(16):
                        nc.sync.dma_start(
                            xts[:, dc, :], xT[dc * 128:(dc + 1) * 128, ssl])
                    kps = pp.tile([128, 512], f32, tag="qk", bufs=2)
                    vps = pp.tile([128, 4, 128], f32, tag="v")
                    for dc in range(16):
                        nc.tensor.matmul(kps[:], wk_sb[:, dc, :],
                                         xts[:, dc, :], start=(dc == 0),
                                         stop=(dc == 15))
                    # one accumulation group per psum zero-region at a time
                    for c in range(4):
                        for dc in range(16):
                            nc.tensor.matmul(
                                vps[:, c, :],
                                xts[:, dc, c * 128:(c + 1) * 128],
                                wv_sb[:, dc, :], start=(dc == 0),
                                stop=(dc == 15))
                    for c in range(4):
                        nc.vector.tensor_copy(V1[:, 4 * sb + c, 0:64],
                                              vps[:, c, 0:64])
                        nc.vector.tensor_copy(V1[:, 4 * sb + c, 66:130],
                                              vps[:, c, 64:128])
                    rope(kps, kT, ssl, k2=True)
                    for qi in range(4):
                        qps = pp.tile([128, 512], f32, tag="qk", bufs=2)
                        for dc in range(16):
                            nc.tensor.matmul(
                                qps[:], wq_sb[:, dc, qi * 128:(qi + 1) * 128],
                                xts[:, dc, :], start=(dc == 0), stop=(dc == 15))
                        rope(qps, qT[qi], ssl)

                    # ---- attention for q block j = sb ----
                    ni = 4 * (sb + 1)
                    for h in range(8):
                        kv = h // 4
                        qt = qT[h // 2]
                        qp = 64 * (h % 2)
                        vcol = slice(0, 65) if kv == 0 else slice(66, 131)
                        ksrc = kT if qp == 64 * kv else kT2
                        kpart = slice(qp, qp + 64)
                        pav = pavp.tile([65, 512], f32, tag="pav")
                        qap = qt[kpart, ssl]
                        for ip in range(ni // 2):
                            ps = psc.tile([128, 2, 512], f32, tag="ps")
                            for b in range(2):
                                i = 2 * ip + b
                                nc.tensor.matmul(
                                    ps[:, b, :],
                                    ksrc[kpart, i * 128:(i + 1) * 128],
                                    qap, start=True, stop=True)
                            et = ep.tile([128, 2, 512], bf16, tag="et")
                            nc.scalar.activation(et[:], ps[:], ACTF.Exp,
                                                 scale=0.125)
                            tp = 2 * ip - 4 * sb
                            if tp >= 0:  # diagonal band: causal mask
                                nc.vector.tensor_mul(
                                    et[:], et[:],
                                    masks_sb[:, tp * 512:(tp + 2) * 512])
                            for b in range(2):
                                i = 2 * ip + b
                                nc.tensor.matmul(pav[:], V1[:, i, vcol],
                                                 et[:, b, :],
                                                 start=(i == 0),
                                                 stop=(i == ni - 1))
                        # normalize by fused denominator (row 64)
                        pavs = nm.tile([65, 512], bf16, tag="pavs")
                        nc.vector.tensor_copy(pavs[:], pav[:])
                        rcp = nm.tile([1, 512], bf16, tag="rcp")
                        with nc.allow_low_precision(reason="softmax denom"):
                            nc.vector.reciprocal(rcp[0:1, :], pavs[64:65, :])
                        bc = nm.tile([64, 512], bf16, tag="bc")
                        nc.gpsimd.partition_broadcast(bc[:], rcp[0:1, :])
                        yt = nm.tile([64, 512], bf16, tag="yt")
                        nc.vector.tensor_mul(yt[:], pavs[0:64, :], bc[:])
                        nc.gpsimd.dma_start(
                            y_loc[sb][h * 64:(h + 1) * 64, :], yt[:])

                    nc.gpsimd.collective_compute(
                        "AllGather", mybir.AluOpType.bypass,
                        ins=[y_loc[sb].opt()], outs=[y_ful[sb].opt()],
                        replica_groups=GROUPS)

            # ---- out projection, one seq block per gathered chunk ----
            with tc.tile_pool(name="yst", bufs=3) as yst, \
                 tc.tile_pool(name="ops", bufs=1, space="PSUM") as op_pool, \
                 tc.tile_pool(name="otp", bufs=2) as otp:
                for j in range(NSB):
                    pso = [op_pool.tile([128, 512], f32, tag=f"o{s4}",
                                        name=f"o{s4}_{j}")
                           for s4 in range(4)]
                    for fc in range(16):
                        yt_t = yst.tile([128, 512], bf16)
                        nc.sync.dma_start(
                            yt_t[:], y_ful[j][fc * 128:(fc + 1) * 128, :])
                        for s4 in range(4):
                            nc.tensor.matmul(
                                pso[s4][:], yt_t[:, s4 * 128:(s4 + 1) * 128],
                                wo_sb[:, fc, :], start=(fc == 0),
                                stop=(fc == 15))
                    for s4 in range(4):
                        ot = otp.tile([128, 512], f32)
                        nc.vector.tensor_copy(ot[:], pso[s4][:])
                        r0 = j * 512 + s4 * 128
                        nc.sync.dma_start(out[r0:r0 + 128, :], ot[:])

    nc.compile()
    return nc


def _host_constants():
    inv_freq = (1.0 / (ROPE_BASE ** (np.arange(0, HEAD_DIM, 2, dtype=np.float32)
                                     / HEAD_DIM))).astype(np.float32)
    t = np.arange(SEQ, dtype=np.float32)
    freqs = np.outer(t, inv_freq)                      # [S, 32]
    emb = np.concatenate([freqs, freqs], axis=-1)      # [S, 64]
    cosT = np.cos(emb).T                               # [64, S]
    sinT = np.sin(emb).T
    # permuted rows + rotation sign folded into sin
    cosP = cosT[PERM]
    sign = np.where((np.arange(64) % 32) < 16, -1.0, 1.0)[:, None]
    sinP = sinT[PERM] * sign
    cosr = np.ascontiguousarray(np.vstack([cosP, cosP])).astype(BF)  # [128,S]
    sinr = np.ascontiguousarray(np.vstack([sinP, sinP])).astype(BF)

    k_idx = np.arange(128)[:, None]
    q_idx = np.arange(512)[None, :]
    m = np.concatenate(
        [(128 * t_ + k_idx <= q_idx).astype(np.float32) for t_ in range(4)],
        axis=1)                                        # [128, 2048]
    return cosr, sinr, np.ascontiguousarray(m).astype(BF)


def _permute_heads(w, nheads):
    # apply PERM within each head's 64 columns
    d = w.shape[0]
    return np.ascontiguousarray(
        w.reshape(d, nheads, HEAD_DIM)[:, :, PERM].reshape(d, nheads * HEAD_DIM))


def _in_maps(x, Wq, Wk, Wv, Wo):
    cosr, sinr, masks = _host_constants()
    xb = [np.ascontiguousarray(x[b].T).astype(BF) for b in range(BATCH)]
    wqb = _permute_heads(Wq, 32).astype(BF)
    wkb = _permute_heads(Wk, 8).astype(BF)
    wvb = Wv.astype(BF)
    wob = Wo.astype(BF)
    maps = []
    for c in range(NCORES):
        b, g = c // 4, c % 4
        maps.append({
            "xT": xb[b],
            "wq": np.ascontiguousarray(wqb[:, g * QF:(g + 1) * QF]),
            "wk": np.ascontiguousarray(wkb[:, g * KF:(g + 1) * KF]),
            "wv": np.ascontiguousarray(wvb[:, g * KF:(g + 1) * KF]),
            "wo": np.ascontiguousarray(wob[:, g * QF:(g + 1) * QF]),
            "cosr": cosr, "sinr": sinr, "masks": masks,
        })
    return maps


def kernel(x, Wq, Wk, Wv, Wo):
    x = np.asarray(x, dtype=np.float32)
    Wq = np.asarray(Wq, dtype=np.float32)
    Wk = np.asarray(Wk, dtype=np.float32)
    Wv = np.asarray(Wv, dtype=np.float32)
    Wo = np.asarray(Wo, dtype=np.float32)

    if "nc" not in _cache:
        _cache["nc"] = _build_kernel()
    nc = _cache["nc"]

    res = run_bass_kernel_spmd(nc, _in_maps(x, Wq, Wk, Wv, Wo),
                               list(range(NCORES)))
    out = np.empty((BATCH, SEQ, MODEL_DIM), dtype=np.float32)
    for c in range(NCORES):
        b, g = c // 4, c % 4
        out[b, :, g * QF:(g + 1) * QF] = res.results[c]["out"]
    return out

